# revision 20
# baseline (speedup 1.0000x reference)
"""EHM (SMPLX body + FLAME head + MANO hands) Bass kernel for 8 TRN2 NeuronCores.

Sharding: VERTEX sharding — each core owns 1/8 of the SMPLX vertices (plus the
FLAME/MANO vertices its SMPLX rows stitch in) and computes ALL B=128 batch
elements for its shard.

Key structure (v2):
  * Joint regression is linear in betas, so J = (J_reg @ [shapedirs|tmpl]) @
    [beta;1] is precomputed on the host into small `jsd` matrices — no
    AllReduce, no per-vertex J partials, and FK runs at the start of the
    kernel fully overlapped with blend-shape matmuls.
  * MANO betas are batch-constant: mano v_shaped / joints / rel-joints are
    host constants.
  * Head-stitched SMPLX rows only need their posedirs term (their blend value
    is replaced), so those chunks skip the shapedirs matmuls.
  * Shapedirs+posedirs accumulate in one PSUM group per chunk.
  * Stitch biases ride as extra rows of the skinning rhs (no bcast matmuls).
  * FK forest is split: SMPLX tree on the vector engine, FLAME+hands on
    gpsimd(Pool); skinning t_apply alternates between the two engines.

Per-vertex data layout: [vertex(partition<=128), (c, b)] with c-major free dim
(col = c*128 + b).  Batch-staged data (poses, FK, A matrices): [b(part), free].
"""

import sys

sys.path.insert(0, "/opt/trn_rl_repo")

from contextlib import ExitStack

import numpy as np
import ml_dtypes

BF16NP = ml_dtypes.bfloat16

import concourse.bass as bass
import concourse.bacc as bacc
import concourse.tile as tile
import concourse.mybir as mybir
from concourse.bass_utils import run_bass_kernel_spmd

F32 = mybir.dt.float32
BF16 = mybir.dt.bfloat16
AF = mybir.ActivationFunctionType
ALU = mybir.AluOpType

# ---------------------------------------------------------------- constants
B = 128
VS, VF, VM = 10475, 5023, 778
NL = 350
NCORES = 8

SMPLX_PARENTS = np.array([-1,0,0,0,1,2,3,4,5,6,7,8,9,9,9,12,13,14,16,17,18,19,
                          15,15,15,20,25,26,20,28,29,20,31,32,20,34,35,20,37,38,
                          21,40,41,21,43,44,21,46,47,21,49,50,21,52,53])
FLAME_PARENTS = np.array([-1,0,1,1,1])
MANO_PARENTS = np.array([-1,0,1,2,0,4,5,0,7,8,0,10,11,0,13,14])

N_PLAIN, N_HEAD, N_HL, N_HR = 768, 384, 128, 128
ROWS = N_PLAIN + N_HEAD + N_HL + N_HR        # 1408
NCH = ROWS // 128                            # 11
CH_PLAIN = set(range(0, 6))
CH_HEAD0 = 6                                 # chunks 6,7,8 head; 9 L; 10 R
CH_HL, CH_HR = 9, 10

PD_S_K = 189
PD_F_K = 27
PD_M_K = 135

NROT = 55
ROT_S0, ROT_F0, ROT_L0, ROT_R0 = 0, 22, 25, 40

# pool-forest joint offsets (flame 0..4, left hand 5..20, right hand 21..36)
PF_F, PF_L, PF_R = 0, 5, 21
NJ_P = 37

BF16_INPUTS = {"w_s", "wre_f", "w_m",
               "sd_s", "pd_s_a", "pd_s_b", "sd_f", "pd_f",
               "pd_m_a", "pd_m_b", "jsds", "jsdf",
               "betaT_s", "betaT_f"}


def _fk_levels(par):
    nj = len(par)
    depth = np.zeros(nj, np.int64)
    for j in range(nj):
        if par[j] >= 0:
            depth[j] = depth[par[j]] + 1
    levels = []
    for d in range(1, int(depth.max()) + 1):
        js = np.nonzero(depth == d)[0]
        runs, i = [], 0
        while i < len(js):
            j0, p0 = int(js[i]), int(par[js[i]])
            if i + 1 < len(js):
                ds = int(js[i + 1]) - j0
                ps = int(par[js[i + 1]]) - p0
            else:
                ds, ps = 1, 0
            n = 1
            while (i + n < len(js) and int(js[i + n]) == j0 + n * ds
                   and int(par[js[i + n]]) == p0 + n * ps):
                n += 1
            if n == 1:
                ds, ps = 1, 0
            runs.append((j0, ds, n, p0, ps))
            i += n
        levels.append(runs)
    return levels


def _par_pool():
    par = np.empty(NJ_P, np.int64)
    par[PF_F:PF_F + 5] = np.where(FLAME_PARENTS < 0, -1, FLAME_PARENTS + PF_F)
    par[PF_L:PF_L + 16] = np.where(MANO_PARENTS < 0, -1, MANO_PARENTS + PF_L)
    par[PF_R:PF_R + 16] = np.where(MANO_PARENTS < 0, -1, MANO_PARENTS + PF_R)
    return par


# ================================================================ host prep

def _split_sizes(total, parts):
    q, r = divmod(total, parts)
    return [q + (1 if i < r else 0) for i in range(parts)]


def _pad_ids(ids, n):
    out = np.full(n, -1, np.int64)
    out[:len(ids)] = ids
    return out


def _mrel(par, nj):
    m = np.eye(nj, dtype=np.float32)
    for j in range(1, nj):
        if par[j] >= 0:
            m[j, par[j]] = -1.0
    return m


def _host_prep(inp):
    f32 = np.float32
    s2f = np.asarray(inp["smplx2flame_ind"])
    head_ix = np.asarray(inp["head_index"])
    s2l = np.asarray(inp["smplx2mano_left"])
    s2r = np.asarray(inp["smplx2mano_right"])

    head_sv = s2f[head_ix]
    special = np.zeros(VS, bool)
    special[head_sv] = True
    special[s2l] = True
    special[s2r] = True
    plain_sv = np.nonzero(~special)[0]

    pl_sp = np.cumsum([0] + _split_sizes(len(plain_sv), NCORES))
    hd_sp = np.cumsum([0] + _split_sizes(len(head_ix), NCORES))
    hl_sp = np.cumsum([0] + _split_sizes(VM, NCORES))

    sd_s_np = np.asarray(inp["smplx_shapedirs"], f32)
    pd_s_np = np.asarray(inp["smplx_posedirs"], f32)
    jr_s_np = np.asarray(inp["smplx_J_regressor"], f32)
    w_s_np = np.asarray(inp["smplx_lbs_weights"], f32)
    tmpl_s = np.asarray(inp["smplx_v_template"], f32)
    sd_f_np = np.asarray(inp["flame_shapedirs"], f32)
    pd_f_np = np.asarray(inp["flame_posedirs"], f32)
    jr_f_np = np.asarray(inp["flame_J_regressor"], f32)
    w_f_np = np.asarray(inp["flame_lbs_weights"], f32)
    tmpl_f = np.asarray(inp["flame_v_template"], f32)
    re_np = np.asarray(inp["r_eyelid"], f32)
    le_np = np.asarray(inp["l_eyelid"], f32)
    sd_m_np = np.asarray(inp["mano_shapedirs"], f32)
    pd_m_np = np.asarray(inp["mano_posedirs"], f32)
    jr_m_np = np.asarray(inp["mano_J_regressor"], f32)
    w_m_np = np.asarray(inp["mano_lbs_weights"], f32)
    tmpl_m = np.asarray(inp["mano_v_template"], f32)

    aa = np.concatenate([
        np.asarray(inp["global_pose"], f32).reshape(B, 3),
        np.asarray(inp["body_pose"], f32).reshape(B, 63),
        np.asarray(inp["jaw_params"], f32).reshape(B, 3),
        np.asarray(inp["eye_pose"], f32).reshape(B, 6),
        np.asarray(inp["left_hand_pose"], f32).reshape(B, 45),
        np.asarray(inp["right_hand_pose"], f32).reshape(B, 45),
    ], axis=1)

    ep = np.asarray(inp["eyelid_params"], f32)
    aux = np.concatenate([
        np.asarray(inp["head_scale"], f32)[:, None],
        np.asarray(inp["left_hand_scale"], f32)[:, None],
        np.asarray(inp["right_hand_scale"], f32)[:, None],
        ep[:, 0:1], ep[:, 1:2],
        np.asarray(inp["head_pos_offset"], f32),
        np.asarray(inp["left_hand_pos_offset"], f32),
        np.asarray(inp["right_hand_pos_offset"], f32),
    ], axis=1)                                               # [128, 14]

    def beta_T(second):
        b = np.concatenate([np.asarray(inp["shape_params"], f32), second], 1)
        bt = np.zeros((384, B), f32)
        bt[:NL] = b.T
        bt[NL] = 1.0
        return bt.reshape(3, 128, B)

    betaT_s = beta_T(np.asarray(inp["body_exp"], f32))
    betaT_f = beta_T(np.asarray(inp["flame_exp"], f32))

    mrel_s = _mrel(SMPLX_PARENTS, 55)
    mrel_f = _mrel(FLAME_PARENTS, 5)
    mrel_m = _mrel(MANO_PARENTS, 16)

    def jsd_slab(jr, sd, tmpl, mrel, nj):
        jsd = np.einsum('jv,vcl->lcj', jr, sd)           # [350, 3, nj]
        jt = jr @ tmpl                                    # [nj, 3]
        jsd_r = np.einsum('lcj,kj->lck', jsd, mrel)
        jt_r = mrel @ jt
        slab = np.zeros((384, 2 * 3 * nj), f32)
        slab[:NL, :3 * nj] = jsd.reshape(NL, 3 * nj)
        slab[NL, :3 * nj] = jt.T.reshape(3 * nj)
        slab[:NL, 3 * nj:] = jsd_r.reshape(NL, 3 * nj)
        slab[NL, 3 * nj:] = jt_r.T.reshape(3 * nj)
        return np.ascontiguousarray(slab.reshape(3, 128, 2 * 3 * nj))

    jsds = jsd_slab(jr_s_np, sd_s_np, tmpl_s, mrel_s, 55)
    jsdf = jsd_slab(jr_f_np, sd_f_np, tmpl_f, mrel_f, 5)

    joff = np.asarray(inp["joints_offset"], f32)          # [B, 55, 3]
    joffB = np.ascontiguousarray(joff.transpose(0, 2, 1).reshape(B, 165))
    joffrelB = np.ascontiguousarray(
        np.einsum('kj,bjc->bck', mrel_s, joff).reshape(B, 165))

    mb = np.asarray(inp["mano_betas"], f32)[0]
    vsh_m = tmpl_m + sd_m_np @ mb                          # [778, 3]
    J_m = jr_m_np @ vsh_m
    rel_m = mrel_m @ J_m
    jmb_c = np.ascontiguousarray(np.broadcast_to(J_m.T.reshape(1, 48), (B, 48)))
    relmb_c = np.ascontiguousarray(np.broadcast_to(rel_m.T.reshape(1, 48), (B, 48)))

    rep = dict(aa=aa, aux=aux, betaT_s=betaT_s, betaT_f=betaT_f,
               joffB=joffB, joffrelB=joffrelB, jmb_c=jmb_c, relmb_c=relmb_c,
               jsds=jsds, jsdf=jsdf, ident=np.eye(128, dtype=f32))

    in_maps = []
    vid_all = np.full((NCORES, ROWS), -1, np.int64)

    for c in range(NCORES):
        p_ids = plain_sv[pl_sp[c]:pl_sp[c + 1]]
        h_pos = np.arange(hd_sp[c], hd_sp[c + 1])
        h_sv, h_fv = head_sv[h_pos], head_ix[h_pos]
        l_pos = np.arange(hl_sp[c], hl_sp[c + 1])
        l_sv, r_sv = s2l[l_pos], s2r[l_pos]

        vid = np.full(ROWS, -1, np.int64)
        vid[:len(p_ids)] = p_ids
        vid[N_PLAIN:N_PLAIN + len(h_sv)] = h_sv
        vid[N_PLAIN + N_HEAD:N_PLAIN + N_HEAD + len(l_sv)] = l_sv
        vid[N_PLAIN + N_HEAD + N_HL:N_PLAIN + N_HEAD + N_HL + len(r_sv)] = r_sv
        vid_all[c] = vid
        vok = vid >= 0
        vc = np.where(vok, vid, 0)

        # smplx shapedirs slab, PLAIN chunks only: [6, 128(l), (c, lk, v)]
        pok, pc = vok[:N_PLAIN], vc[:N_PLAIN]
        sdp = np.zeros((N_PLAIN, 3, 384), f32)
        sdp[:, :, :NL] = np.where(pok[:, None, None], sd_s_np[pc], 0.0)
        sdp[:, :, NL] = np.where(pok[:, None], tmpl_s[pc], 0.0)
        slab = sdp.reshape(6, 128, 3, 3, 128).transpose(0, 4, 2, 3, 1)
        sd_s = np.ascontiguousarray(slab).reshape(6, 128, 1152)

        colv = vc[:, None] * 3 + np.arange(3)[None, :]
        pdv = pd_s_np[:PD_S_K][:, colv]
        pdv = np.where(vok[None, :, None], pdv, 0.0)
        pdv = pdv.reshape(PD_S_K, NCH, 128, 3).transpose(1, 0, 3, 2)
        pd_s_a = np.ascontiguousarray(pdv[:, :128]).reshape(NCH, 128, 384)
        pd_s_b = np.ascontiguousarray(pdv[:, 128:]).reshape(NCH, PD_S_K - 128, 384)

        w_s = np.ascontiguousarray(
            np.where(vok[:, None], w_s_np[vc], 0.0)
            .reshape(NCH, 128, 55).transpose(0, 2, 1))

        # flame gathered chunks
        fg = _pad_ids(h_fv, N_HEAD)
        fok = fg >= 0
        fc = np.where(fok, fg, 0)
        sdfp = np.zeros((N_HEAD, 3, 384), f32)
        sdfp[:, :, :NL] = np.where(fok[:, None, None], sd_f_np[fc], 0.0)
        sdfp[:, :, NL] = np.where(fok[:, None], tmpl_f[fc], 0.0)
        slab = sdfp.reshape(3, 128, 3, 3, 128).transpose(0, 4, 2, 3, 1)
        sd_f = np.ascontiguousarray(slab).reshape(3, 128, 1152)

        colf = fc[:, None] * 3 + np.arange(3)[None, :]
        pdfv = pd_f_np[9:36][:, colf]
        pdfv = np.where(fok[None, :, None], pdfv, 0.0)
        pdfv = pdfv.reshape(PD_F_K, 3, 128, 3).transpose(1, 0, 3, 2)
        pd_f = np.ascontiguousarray(pdfv).reshape(3, PD_F_K, 384)

        wre = np.zeros((3, 12, 128), f32)
        for k in range(3):
            rows, ok = fc[k * 128:(k + 1) * 128], fok[k * 128:(k + 1) * 128]
            wre[k, :5] = np.where(ok[None, :], w_f_np[rows].T, 0.0)
            wre[k, 5:8] = np.where(ok[None, :], re_np[rows].T, 0.0)
            wre[k, 8:11] = np.where(ok[None, :], le_np[rows].T, 0.0)
            wre[k, 11] = np.where(ok, 1.0, 0.0)

        # mano (left/right share vertex ids -> shared pd, w, vshm)
        m_rows = _pad_ids(l_pos, 128)
        mok = m_rows >= 0
        mc = np.where(mok, m_rows, 0)
        colm = mc[:, None] * 3 + np.arange(3)[None, :]
        pdm = pd_m_np[:, colm]
        pdm = np.where(mok[None, :, None], pdm, 0.0).transpose(0, 2, 1)
        pd_m_a = np.ascontiguousarray(pdm[:128].reshape(128, 384))
        pd_m_b = np.ascontiguousarray(pdm[128:].reshape(PD_M_K - 128, 384))
        w_m = np.zeros((17, 128), f32)
        w_m[:16] = np.where(mok[None, :], w_m_np[mc].T, 0.0)
        w_m[16] = np.where(mok, 1.0, 0.0)
        vshm = np.where(mok[:, None], vsh_m[mc], 0.0).astype(f32)  # [128, 3]

        m = dict(rep)
        m.update(sd_s=sd_s, pd_s_a=pd_s_a, pd_s_b=pd_s_b, w_s=w_s,
                 sd_f=sd_f, pd_f=pd_f, wre_f=wre,
                 pd_m_a=pd_m_a, pd_m_b=pd_m_b, w_m=w_m, vshm=vshm)
        out = {}
        for k, v in m.items():
            if k in BF16_INPUTS:
                out[k] = np.ascontiguousarray(v.astype(BF16NP))
            else:
                out[k] = np.ascontiguousarray(v, f32)
        in_maps.append(out)

    return in_maps, vid_all


# ================================================================ device IR

def _build_nc():
    nc = bacc.Bacc("TRN2", target_bir_lowering=False, debug=False,
                   num_devices=NCORES)
    di = {}

    def din(name, shape):
        dt = BF16 if name in BF16_INPUTS else F32
        di[name] = nc.dram_tensor(name, list(shape), dt, kind="ExternalInput").ap()

    din("aa", (B, 165)); din("aux", (B, 14))
    din("betaT_s", (3, 128, 128)); din("betaT_f", (3, 128, 128))
    din("joffB", (B, 165)); din("joffrelB", (B, 165))
    din("jmb_c", (B, 48)); din("relmb_c", (B, 48))
    din("jsds", (3, 128, 330)); din("jsdf", (3, 128, 30))
    din("ident", (128, 128))
    din("sd_s", (6, 128, 1152))
    din("pd_s_a", (NCH, 128, 384)); din("pd_s_b", (NCH, PD_S_K - 128, 384))
    din("w_s", (NCH, 55, 128))
    din("sd_f", (3, 128, 1152)); din("pd_f", (3, PD_F_K, 384))
    din("wre_f", (3, 12, 128))
    din("pd_m_a", (128, 384)); din("pd_m_b", (PD_M_K - 128, 384))
    din("w_m", (17, 128)); din("vshm", (128, 3))

    out_d = nc.dram_tensor("out", [ROWS, 384], F32, kind="ExternalOutput").ap()
    dbg_d = None
    if DEBUG:
        dbg_d = nc.dram_tensor("dbg", [128, 4096], F32, kind="ExternalOutput").ap()

    with tile.TileContext(nc) as tc:
        _emit(nc, tc, di, out_d, dbg_d)
    nc.compile()
    return nc


def _emit(nc, tc, di, out_d, dbg_d=None):
    par_p = _par_pool()
    levels_s = _fk_levels(SMPLX_PARENTS)
    levels_p = _fk_levels(par_p)
    es = ExitStack()
    persist = es.enter_context(tc.tile_pool(name="persist", bufs=1))
    slabs = es.enter_context(tc.tile_pool(name="slabs", bufs=3))
    acc_cm = tc.tile_pool(name="acc", bufs=2, space="PSUM")
    acc = acc_cm.__enter__()
    tpz_cm = tc.tile_pool(name="tpz", bufs=2, space="PSUM")
    tpz = tpz_cm.__enter__()

    V, S, G, T, DMA = nc.vector, nc.scalar, nc.gpsimd, nc.tensor, nc.sync

    def ptile(shape, name, dt=F32):
        return persist.tile(list(shape), dt, tag=name, name=name)

    # ---------------- staged inputs (latency-critical first) --------------
    aa = ptile((B, 165), "aa"); DMA.dma_start(aa[:], di["aa"][:])
    aux = ptile((B, 14), "aux"); DMA.dma_start(aux[:], di["aux"][:])
    betaT_s = ptile((128, 384), "betaT_s", BF16)
    betaT_f = ptile((128, 384), "betaT_f", BF16)
    jsds = ptile((128, 3 * 330), "jsds", BF16)
    jsdf = ptile((128, 3 * 30), "jsdf", BF16)
    for lk in range(3):
        DMA.dma_start(betaT_s[:, lk * 128:(lk + 1) * 128], di["betaT_s"][lk])
        DMA.dma_start(jsds[:, lk * 330:(lk + 1) * 330], di["jsds"][lk])
    for lk in range(3):
        DMA.dma_start(betaT_f[:, lk * 128:(lk + 1) * 128], di["betaT_f"][lk])
        DMA.dma_start(jsdf[:, lk * 30:(lk + 1) * 30], di["jsdf"][lk])
    joffB = ptile((B, 165), "joffB"); DMA.dma_start(joffB[:], di["joffB"][:])
    joffrelB = ptile((B, 165), "joffrelB")
    DMA.dma_start(joffrelB[:], di["joffrelB"][:])
    ident = ptile((128, 128), "ident")
    DMA.dma_start(ident[:], di["ident"][:])
    jmb = ptile((B, 48), "jmb"); DMA.dma_start(jmb[:], di["jmb_c"][:])
    relmb = ptile((B, 48), "relmb"); DMA.dma_start(relmb[:], di["relmb_c"][:])
    vshm = ptile((128, 3), "vshm"); DMA.dma_start(vshm[:], di["vshm"][:])

    # warm the scalar-engine activation tables before real work needs them
    warm = ptile((B, 1), "warm")
    warm2 = ptile((B, 1), "warm2")
    nc.gpsimd.memset(warm[:], 0.25)
    S.activation(warm2[:], warm[:], AF.Sqrt, bias=warm[:])
    S.activation(warm2[:], warm[:], AF.Sin, bias=warm[:])

    # ---------------- rodrigues ------------------------------------------
    rot = ptile((B, NROT * 9), "rot")
    _rodrigues(nc, aa, rot, ptile)
    rot4 = rot[:].rearrange("p (j x) -> p j x", x=9)

    def pf_make(name, j0, n):
        t = ptile((B, n * 9), name)
        t9 = t[:].rearrange("p (j x) -> p j x", x=9)
        V.tensor_copy(t9, rot4[:, j0:j0 + n, :])
        V.tensor_scalar_add(t9[:, :, 0:9:4], t9[:, :, 0:9:4], -1.0)
        return t

    pf_s = pf_make("pf_s", 1, 21)
    pf_f = pf_make("pf_f", 22, 3)
    pf_m = [pf_make("pf_l", 25, 15), pf_make("pf_r", 40, 15)]
    epp = ptile((B, 2), "epp")
    V.tensor_mul(epp[:], aux[:, 3:5], aux[:, 0:1].broadcast_to([B, 2]))

    def transpose_to(dst_ap, src_ap):
        pp = tpz.tile([128, 512], F32, tag="tpose")
        k, n = src_ap.shape[0], src_ap.shape[1]
        T.matmul(pp[:n, :k], src_ap, ident[:k, :k], is_transpose=True,
                 start=True, stop=True)
        S.copy(dst_ap, pp[:n, :k])

    pfT_s_a = ptile((128, 128), "pfT_s_a", BF16)
    pfT_s_b = ptile((PD_S_K - 128, 128), "pfT_s_b", BF16)
    transpose_to(pfT_s_a[:], pf_s[:, 0:128])
    transpose_to(pfT_s_b[:], pf_s[:, 128:PD_S_K])
    pfT_f = ptile((PD_F_K, 128), "pfT_f", BF16)
    transpose_to(pfT_f[:], pf_f[:, :])
    pfT_m_a = [ptile((128, 128), "pfT_l_a", BF16), ptile((128, 128), "pfT_r_a", BF16)]
    pfT_m_b = [ptile((PD_M_K - 128, 128), "pfT_l_b", BF16),
               ptile((PD_M_K - 128, 128), "pfT_r_b", BF16)]
    for h in range(2):
        transpose_to(pfT_m_a[h][:], pf_m[h][:, 0:128])
        transpose_to(pfT_m_b[h][:], pf_m[h][:, 128:PD_M_K])
    epT = ptile((2, 128), "epT", BF16)
    transpose_to(epT[:], epp[:, :])

    # ---------------- J from betas (host-precomputed jsd) -----------------
    jsp = acc.tile([128, 512], F32, tag="vppsum", name="jsp")
    for lk in range(3):
        T.matmul(jsp[:, 0:330], betaT_s[:, lk * 128:(lk + 1) * 128],
                 jsds[:, lk * 330:(lk + 1) * 330],
                 start=(lk == 0), stop=(lk == 2))
    jfp = acc.tile([128, 512], F32, tag="vppsum", name="jfp")
    for lk in range(3):
        T.matmul(jfp[:, 0:30], betaT_f[:, lk * 128:(lk + 1) * 128],
                 jsdf[:, lk * 30:(lk + 1) * 30],
                 start=(lk == 0), stop=(lk == 2))

    jb = ptile((B, 165), "jb")
    relb = ptile((B, 165), "relb")
    V.tensor_add(jb[:], jsp[:, 0:165], joffB[:])
    V.tensor_add(relb[:], jsp[:, 165:330], joffrelB[:])
    jfb = ptile((B, 15), "jfb")
    relfb = ptile((B, 15), "relfb")
    S.copy(jfb[:], jfp[:, 0:15])
    S.copy(relfb[:], jfp[:, 15:30])

    # ---------------- FK: smplx on vector, flame+hands on pool ------------
    Tb_s = ptile((B, 55 * 12), "Tb_s")
    Ab_s = ptile((B, 55 * 12), "Ab_s")
    T4s = Tb_s[:].rearrange("p (j m n) -> p j m n", m=3, n=4)
    A4s = Ab_s[:].rearrange("p (j m n) -> p j m n", m=3, n=4)
    Tb_p = ptile((B, NJ_P * 12), "Tb_p")
    Ab_p = ptile((B, NJ_P * 12), "Ab_p")
    T4p = Tb_p[:].rearrange("p (j m n) -> p j m n", m=3, n=4)
    A4p = Ab_p[:].rearrange("p (j m n) -> p j m n", m=3, n=4)

    V.memset(Tb_s[:], 0.0)
    G.memset(Tb_p[:], 0.0)
    V.tensor_copy(T4s[:, 0:22, :, 0:3],
                  rot4[:, 0:22, :].rearrange("p j (m n) -> p j m n", n=3))
    V.memset(Tb_s[:].rearrange("p (j x) -> p j x", x=12)[:, 22:55, 0:11:5], 1.0)
    V.tensor_copy(T4s[:, 0:55, :, 3], relb[:].rearrange("p (c j) -> p j c", c=3))

    G.tensor_copy(T4p[:, PF_F + 2:PF_F + 5, :, 0:3],
                  rot4[:, ROT_F0:ROT_F0 + 3, :].rearrange("p j (m n) -> p j m n", n=3))
    G.tensor_copy(T4p[:, PF_L + 1:PF_L + 16, :, 0:3],
                  rot4[:, ROT_L0:ROT_L0 + 15, :].rearrange("p j (m n) -> p j m n", n=3))
    G.tensor_copy(T4p[:, PF_R + 1:PF_R + 16, :, 0:3],
                  rot4[:, ROT_R0:ROT_R0 + 15, :].rearrange("p j (m n) -> p j m n", n=3))
    for j0, n in ((PF_F, 2), (PF_L, 1), (PF_R, 1)):
        G.memset(Tb_p[:].rearrange("p (j x) -> p j x", x=12)[:, j0:j0 + n, 0:11:5], 1.0)
    G.tensor_copy(T4p[:, PF_F:PF_F + 5, :, 3],
                  relfb[:].rearrange("p (c j) -> p j c", c=3))
    for off in (PF_L, PF_R):
        G.tensor_copy(T4p[:, off:off + 16, :, 3],
                      relmb[:].rearrange("p (c j) -> p j c", c=3))

    def fk_forest(E, T4, A4, levels, roots, tmp_t, tmp2_t):
        for r in roots:
            E.tensor_copy(A4[:, r:r + 1], T4[:, r:r + 1])
        for runs in levels:
            for (d0, ds, n, p0, ps) in runs:
                sl_d = slice(d0, d0 + (n - 1) * ds + 1, ds) if ds != 1 else slice(d0, d0 + n)
                dst, dT = A4[:, sl_d], T4[:, sl_d]
                if ps == 0:
                    par = A4[:, p0:p0 + 1].broadcast_to([B, n, 3, 4])
                else:
                    sl_p = slice(p0, p0 + (n - 1) * ps + 1, ps) if ps != 1 else slice(p0, p0 + n)
                    par = A4[:, sl_p]
                tmp = tmp_t[:].rearrange("p (j m n) -> p j m n", m=3, n=4)[:, :n]
                sc2 = tmp2_t[:].rearrange("p (j m n) -> p j m n", m=3, n=4)[:, :n]
                for k in range(3):
                    a_k = par[:, :, :, k:k + 1].broadcast_to([B, n, 3, 4])
                    t_k = dT[:, :, k:k + 1, :].broadcast_to([B, n, 3, 4])
                    if k == 0:
                        E.tensor_mul(tmp, a_k, t_k)
                    else:
                        E.tensor_mul(sc2, a_k, t_k)
                        E.tensor_add(tmp, tmp, sc2)
                E.tensor_add(tmp[:, :, :, 3], tmp[:, :, :, 3], par[:, :, :, 3])
                E.tensor_copy(dst, tmp)

    fk_tmp_s = ptile((B, 12 * 16), "fk_tmp_s")
    fk_tmp2_s = ptile((B, 12 * 16), "fk_tmp2_s")
    fk_tmp_p = ptile((B, 12 * 16), "fk_tmp_p")
    fk_tmp2_p = ptile((B, 12 * 16), "fk_tmp2_p")
    fk_forest(V, T4s, A4s, levels_s, (0,), fk_tmp_s, fk_tmp2_s)
    fk_forest(G, T4p, A4p, levels_p, (PF_F, PF_L, PF_R), fk_tmp_p, fk_tmp2_p)

    # ---- stitch biases (world translations BEFORE rel-correction) --------
    hm = ptile((B, 16), "hm")
    jb3 = jb[:].rearrange("p (c j) -> p c j", c=3)
    jm3 = jmb[:].rearrange("p (c j) -> p c j", c=3)
    bias9 = ptile((B, 9), "bias9")
    V.tensor_add(hm[:, 0:3], jb3[:, :, 23], jb3[:, :, 24])
    V.tensor_add(hm[:, 3:6], A4p[:, PF_F + 3, :, 3], A4p[:, PF_F + 4, :, 3])
    V.tensor_sub(hm[:, 6:9], hm[:, 0:3], hm[:, 3:6])
    V.tensor_scalar_mul(hm[:, 6:9], hm[:, 6:9], 0.5)
    V.tensor_add(bias9[:, 0:3], hm[:, 6:9], aux[:, 5:8])
    V.tensor_sub(hm[:, 9:12], aux[:, 8:11], jm3[:, :, 0])
    V.tensor_sub(bias9[:, 3:4], jb3[:, 0:1, 20], hm[:, 9:10])
    V.tensor_add(bias9[:, 4:6], hm[:, 10:12], jb3[:, 1:3, 20])
    V.tensor_sub(hm[:, 12:15], aux[:, 11:14], jm3[:, :, 0])
    V.tensor_add(bias9[:, 6:9], hm[:, 12:15], jb3[:, :, 21])

    # ---- A_rel: translation -= R_world @ J ------------------------------
    corr_s = ptile((B, 55 * 3), "corr_s")
    corr_s2 = ptile((B, 55 * 3), "corr_s2")
    corr_p = ptile((B, 16 * 3), "corr_p")
    corr_p2 = ptile((B, 16 * 3), "corr_p2")

    def corr(E, A4, j0, nj, jsrc, ct_t, ct2_t):
        ct = ct_t[:].rearrange("p (j m) -> p j m", m=3)[:, 0:nj]
        ct2 = ct2_t[:].rearrange("p (j m) -> p j m", m=3)[:, 0:nj]
        js = jsrc.rearrange("p (c j) -> p c j", c=3)
        for k in range(3):
            a_k = A4[:, j0:j0 + nj, :, k]
            j_k = js[:, k, :].unsqueeze(2).broadcast_to([B, nj, 3])
            if k == 0:
                E.tensor_mul(ct, a_k, j_k)
            else:
                E.tensor_mul(ct2, a_k, j_k)
                E.tensor_add(ct, ct, ct2)
        E.tensor_sub(A4[:, j0:j0 + nj, :, 3], A4[:, j0:j0 + nj, :, 3], ct)

    corr(V, A4s, 0, 55, jb[:], corr_s, corr_s2)
    corr(G, A4p, PF_F, 5, jfb[:], corr_p, corr_p2)
    corr(G, A4p, PF_L, 16, jmb[:], corr_p, corr_p2)
    corr(G, A4p, PF_R, 16, jmb[:], corr_p, corr_p2)

    # ---- scale folding (pool tile) --------------------------------------
    G.tensor_scalar_mul(Ab_p[:, PF_F * 12:(PF_F + 5) * 12],
                        Ab_p[:, PF_F * 12:(PF_F + 5) * 12], aux[:, 0:1])
    negls = ptile((B, 1), "negls")
    G.tensor_scalar_mul(negls[:], aux[:, 1:2], -1.0)
    AL = A4p[:, PF_L:PF_L + 16]
    G.tensor_scalar_mul(AL[:, :, 0, :], AL[:, :, 0, :], negls[:, 0:1])
    G.tensor_scalar_mul(AL[:, :, 1:3, :], AL[:, :, 1:3, :], aux[:, 1:2])
    ARr = A4p[:, PF_R:PF_R + 16]
    G.tensor_scalar_mul(ARr[:, :, :, :], ARr[:, :, :, :], aux[:, 2:3])

    # ================= blend chunks (tensor) ==============================
    vp_sbuf = [ptile((128, 384), f"vp{i}") for i in range(NCH)]
    vpf_sbuf = [ptile((128, 384), f"vpf{h}") for h in range(3)]
    vpm_sbuf = [ptile((128, 384), f"vpm{h}") for h in range(2)]

    def blend_smplx(i):
        pda = slabs.tile((128, 384), BF16, tag="pd_s_a")
        pdb = slabs.tile((PD_S_K - 128, 384), BF16, tag="pd_s_b")
        DMA.dma_start(pda[:], di["pd_s_a"][i])
        DMA.dma_start(pdb[:], di["pd_s_b"][i])
        sdt = None
        if i in CH_PLAIN:
            sdt = slabs.tile((128, 1152), BF16, tag="sd_s")
            DMA.dma_start(sdt[:], di["sd_s"][i])
        pp = acc.tile([128, 512], F32, tag="vppsum")
        # one accumulation group per c3 (interleaved groups within a PSUM
        # bank corrupt accumulation on HW)
        for c3 in range(3):
            first = True
            if sdt is not None:
                for lk in range(3):
                    T.matmul(pp[:, c3 * 128:(c3 + 1) * 128],
                             sdt[:, (c3 * 3 + lk) * 128:(c3 * 3 + lk + 1) * 128],
                             betaT_s[:, lk * 128:(lk + 1) * 128],
                             start=first, stop=False)
                    first = False
            T.matmul(pp[:, c3 * 128:(c3 + 1) * 128],
                     pda[:, c3 * 128:(c3 + 1) * 128], pfT_s_a[:],
                     start=first, stop=False)
            T.matmul(pp[:, c3 * 128:(c3 + 1) * 128],
                     pdb[:, c3 * 128:(c3 + 1) * 128], pfT_s_b[:],
                     start=False, stop=True)
        S.copy(vp_sbuf[i][:], pp[:, 0:384])

    def blend_flame(h):
        sdt = slabs.tile((128, 1152), BF16, tag="sd_f")
        DMA.dma_start(sdt[:], di["sd_f"][h])
        pdf = slabs.tile((PD_F_K, 384), BF16, tag="pd_f")
        DMA.dma_start(pdf[:], di["pd_f"][h])
        pp = acc.tile([128, 512], F32, tag="vppsum")
        for c3 in range(3):
            for lk in range(3):
                T.matmul(pp[:, c3 * 128:(c3 + 1) * 128],
                         sdt[:, (c3 * 3 + lk) * 128:(c3 * 3 + lk + 1) * 128],
                         betaT_f[:, lk * 128:(lk + 1) * 128],
                         start=(lk == 0), stop=False)
            T.matmul(pp[:, c3 * 128:(c3 + 1) * 128],
                     pdf[:, c3 * 128:(c3 + 1) * 128], pfT_f[:],
                     start=False, stop=True)
        S.copy(vpf_sbuf[h][:], pp[:, 0:384])

    pdm_a = ptile((128, 384), "pdm_a", BF16)
    pdm_b = ptile((PD_M_K - 128, 384), "pdm_b", BF16)
    DMA.dma_start(pdm_a[:], di["pd_m_a"][:])
    DMA.dma_start(pdm_b[:], di["pd_m_b"][:])

    def blend_mano(h):
        pp = acc.tile([128, 512], F32, tag="vppsum")
        for c3 in range(3):
            T.matmul(pp[:, c3 * 128:(c3 + 1) * 128],
                     pdm_a[:, c3 * 128:(c3 + 1) * 128], pfT_m_a[h][:],
                     start=True, stop=False)
            T.matmul(pp[:, c3 * 128:(c3 + 1) * 128],
                     pdm_b[:, c3 * 128:(c3 + 1) * 128], pfT_m_b[h][:],
                     start=False, stop=True)
        vpm = vpm_sbuf[h]
        V.tensor_add(vpm[:].rearrange("p (c b) -> p c b", b=128),
                     pp[:, 0:384].rearrange("p (c b) -> p c b", b=128),
                     vshm[:].unsqueeze(2).broadcast_to([128, 3, 128]))

    for i in range(5):
        blend_smplx(i)

    # ---- rhs assembly (A matrices are ready by now) ----------------------
    def rhs_fill(rhs_t, A4, j0, nj, col0, n4):
        pp = tpz.tile([128, 512], F32, tag="tpose")
        for m3 in range(3):
            T.matmul(pp[0:nj, m3 * 128:(m3 + 1) * 128],
                     A4[:, j0:j0 + nj, m3, n4], ident[:],
                     is_transpose=True, start=True, stop=True)
        S.copy(rhs_t[0:nj, col0:col0 + 384], pp[0:nj, 0:384])

    rhs_s = persist.tile([55, 1536], BF16, tag="rhs_s", name="rhs_s")
    for n4 in range(4):
        rhs_fill(rhs_s, A4s, 0, 55, n4 * 384, n4)

    # eyelid and stitch-bias terms are purely additive -> they ride as extra
    # rows of the n=3 (translation) column group of the skinning rhs.
    rhs_f = persist.tile([12, 1536], BF16, tag="rhs_f", name="rhs_f")
    G.memset(rhs_f[:], 0.0)
    for n4 in range(4):
        rhs_fill(rhs_f, A4p, PF_F, 5, n4 * 384, n4)

    bias9T = ptile((9, 128), "bias9T", BF16)
    transpose_to(bias9T[:], bias9[:, :])
    for m3 in range(3):
        DMA.dma_start(rhs_f[5 + m3:6 + m3, 1152 + m3 * 128:1280 + m3 * 128],
                      epT[1:2, :])
        DMA.dma_start(rhs_f[8 + m3:9 + m3, 1152 + m3 * 128:1280 + m3 * 128],
                      epT[0:1, :])
        DMA.dma_start(rhs_f[11:12, 1152 + m3 * 128:1280 + m3 * 128],
                      bias9T[m3:m3 + 1, :])

    rhs_m = [persist.tile([17, 1536], BF16, tag="rhs_l", name="rhs_l"),
             persist.tile([17, 1536], BF16, tag="rhs_r", name="rhs_r")]
    for h, off in ((0, PF_L), (1, PF_R)):
        G.memset(rhs_m[h][:], 0.0)
        for n4 in range(4):
            rhs_fill(rhs_m[h], A4p, off, 16, n4 * 384, n4)
        for m3 in range(3):
            DMA.dma_start(rhs_m[h][16:17, 1152 + m3 * 128:1280 + m3 * 128],
                          bias9T[3 * (h + 1) + m3:3 * (h + 1) + m3 + 1, :])

    if dbg_d is not None:
        DMA.dma_start(dbg_d[0:12, 0:1536], rhs_f[:])
        DMA.dma_start(dbg_d[16:33, 0:1536], rhs_m[0][:])
        DMA.dma_start(dbg_d[40:95, 0:1536], rhs_s[:])
        DMA.dma_start(dbg_d[100:101, 0:165], jb[0:1, :])
        DMA.dma_start(dbg_d[101:102, 0:165], relb[0:1, :])
        DMA.dma_start(dbg_d[102:103, 0:15], jfb[0:1, :])

    # ================= skinning, interleaved with remaining blends ========
    tpz_cm.__exit__(None, None, None)
    big_cm = tc.tile_pool(name="big", bufs=2, space="PSUM")
    big = big_cm.__enter__()

    def t_apply(E, dst_ap, tp_ap, x_sbuf, scratch):
        """dst = sum_{n<3} T'[n]*x_n + T'[3]; layouts (n, m, b)."""
        d3 = dst_ap.rearrange("p (m b) -> p m b", b=128)
        x3 = x_sbuf[:].rearrange("p (c b) -> p c b", b=128)
        tp = tp_ap.rearrange("p (n m b) -> p n m b", m=3, b=128)
        sc = scratch.rearrange("p (m b) -> p m b", b=128)
        E.tensor_mul(d3, tp[:, 0], x3[:, 0:1].broadcast_to([128, 3, 128]))
        for n4 in (1, 2):
            E.tensor_mul(sc, tp[:, n4], x3[:, n4:n4 + 1].broadcast_to([128, 3, 128]))
            E.tensor_add(d3, d3, sc)
        E.tensor_add(d3, d3, tp[:, 3])

    scr_v = [ptile((128, 384), f"scrv{i}") for i in range(2)]
    scr_g = [ptile((128, 384), f"scrg{i}") for i in range(2)]
    cnt_v, cnt_g = [0], [0]

    def skin_mm(wt, rhs_t):
        tp = big.tile([128, 1536], F32, tag="bigp")
        for g in range(3):
            T.matmul(tp[:, g * 512:(g + 1) * 512], wt[:],
                     rhs_t[:, g * 512:(g + 1) * 512], start=True, stop=True)
        return tp

    def apply_pool(dst_ap, tp_psum, x_sbuf):
        # gpsimd cannot read PSUM: bounce T' through SBUF on the scalar engine
        tpsb = slabs.tile((128, 1536), F32, tag="tpsb", bufs=2, name="tpsb")
        S.copy(tpsb[:], tp_psum[:])
        cnt_g[0] += 1
        t_apply(G, dst_ap, tpsb[:], x_sbuf, scr_g[cnt_g[0] % 2][:])

    def apply_vec(dst_ap, tp_psum, x_sbuf):
        cnt_v[0] += 1
        t_apply(V, dst_ap, tp_psum[:], x_sbuf, scr_v[cnt_v[0] % 2][:])

    def skin_chunk(i):
        if CH_HEAD0 <= i < CH_HEAD0 + 3:
            h = i - CH_HEAD0
            hv = slabs.tile((128, 384), F32, tag="hv", bufs=2, name="hv")
            wt = slabs.tile((12, 128), BF16, tag="wre_f")
            DMA.dma_start(wt[:], di["wre_f"][h])
            tp1 = skin_mm(wt, rhs_f)
            apply_pool(hv[:], tp1, vpf_sbuf[h])
            G.tensor_add(vp_sbuf[i][:], vp_sbuf[i][:], hv[:])
        elif i in (CH_HL, CH_HR):
            h = i - CH_HL
            hv = slabs.tile((128, 384), F32, tag="hv", bufs=2, name="hv")
            wt = slabs.tile((17, 128), BF16, tag="w_m")
            DMA.dma_start(wt[:], di["w_m"][:])
            tpm = skin_mm(wt, rhs_m[h])
            apply_pool(hv[:], tpm, vpm_sbuf[h])
            G.tensor_add(vp_sbuf[i][:], vp_sbuf[i][:], hv[:])

        wt = slabs.tile((55, 128), BF16, tag="w_s")
        DMA.dma_start(wt[:], di["w_s"][i])
        tps = skin_mm(wt, rhs_s)
        ot = slabs.tile((128, 384), F32, tag="outt", bufs=3, name="ot")
        if i in CH_PLAIN and i % 2 == 1:
            apply_pool(ot[:], tps, vp_sbuf[i])
        else:
            apply_vec(ot[:], tps, vp_sbuf[i])
        DMA.dma_start(out_d[i * 128:(i + 1) * 128, :], ot[:])

    for i in range(6):
        skin_chunk(i)
        blend_smplx(i + 5)
    for h in range(3):
        blend_flame(h)
    for h in range(2):
        blend_mano(h)
    for i in range(6, NCH):
        skin_chunk(i)

    big_cm.__exit__(None, None, None)
    acc_cm.__exit__(None, None, None)
    es.close()


def _rodrigues(nc, aa, rot, ptile):
    V, S = nc.vector, nc.scalar
    J = NROT
    aa3 = aa[:].rearrange("p (j k) -> p j k", k=3)
    sq = ptile((B, J), "rg_sq")
    tmp = ptile((B, J), "rg_tmp")
    V.tensor_mul(sq[:], aa3[:, :, 0], aa3[:, :, 0])
    V.tensor_mul(tmp[:], aa3[:, :, 1], aa3[:, :, 1])
    V.tensor_add(sq[:], sq[:], tmp[:])
    V.tensor_mul(tmp[:], aa3[:, :, 2], aa3[:, :, 2])
    V.tensor_add(sq[:], sq[:], tmp[:])
    eps_t = ptile((B, 1), "rg_eps")
    nc.gpsimd.memset(eps_t[:], 1e-8)
    hpi_t = ptile((B, 1), "rg_hpi")
    nc.gpsimd.memset(hpi_t[:], float(np.pi / 2))
    zero_t = ptile((B, 1), "rg_zero")
    nc.gpsimd.memset(zero_t[:], 0.0)
    ang = ptile((B, J), "rg_ang")
    S.activation(ang[:], sq[:], AF.Sqrt, bias=eps_t[:])
    inv = ptile((B, J), "rg_inv")
    V.reciprocal(inv[:], ang[:])
    sn = ptile((B, J), "rg_sin")
    co = ptile((B, J), "rg_cos")
    S.activation(sn[:], ang[:], AF.Sin, bias=zero_t[:])
    S.activation(co[:], ang[:], AF.Sin, bias=hpi_t[:])
    nv = ptile((B, 3 * J), "rg_n")
    n3 = nv[:].rearrange("p (j k) -> p j k", k=3)
    V.tensor_mul(n3, aa3, inv[:].unsqueeze(2).broadcast_to([B, J, 3]))
    u = ptile((B, J), "rg_u")
    V.tensor_scalar(u[:], co[:], -1.0, 1.0, ALU.mult, ALU.add)
    un = ptile((B, 3 * J), "rg_un")
    un3 = un[:].rearrange("p (j k) -> p j k", k=3)
    V.tensor_mul(un3, n3, u[:].unsqueeze(2).broadcast_to([B, J, 3]))
    q = ptile((B, 3 * J), "rg_q")
    q3 = q[:].rearrange("p (j k) -> p j k", k=3)
    V.tensor_mul(q3, un3, n3)
    d = ptile((B, J), "rg_d")
    V.tensor_add(d[:], q3[:, :, 0], q3[:, :, 1])
    V.tensor_add(d[:], d[:], q3[:, :, 2])
    dd = ptile((B, J), "rg_dd")
    V.tensor_scalar(dd[:], d[:], -1.0, 1.0, ALU.mult, ALU.add)
    snv = ptile((B, 3 * J), "rg_snv")
    s3 = snv[:].rearrange("p (j k) -> p j k", k=3)
    V.tensor_mul(s3, n3, sn[:].unsqueeze(2).broadcast_to([B, J, 3]))
    r4 = rot[:].rearrange("p (j m n) -> p j m n", m=3, n=3)
    for m in range(3):
        V.tensor_add(r4[:, :, m, m], q3[:, :, m], dd[:])
    p = ptile((B, J), "rg_p")
    V.tensor_mul(p[:], un3[:, :, 0], n3[:, :, 1])
    V.tensor_sub(r4[:, :, 0, 1], p[:], s3[:, :, 2])
    V.tensor_add(r4[:, :, 1, 0], p[:], s3[:, :, 2])
    V.tensor_mul(p[:], un3[:, :, 0], n3[:, :, 2])
    V.tensor_add(r4[:, :, 0, 2], p[:], s3[:, :, 1])
    V.tensor_sub(r4[:, :, 2, 0], p[:], s3[:, :, 1])
    V.tensor_mul(p[:], un3[:, :, 1], n3[:, :, 2])
    V.tensor_sub(r4[:, :, 1, 2], p[:], s3[:, :, 0])
    V.tensor_add(r4[:, :, 2, 1], p[:], s3[:, :, 0])


# ================================================================ entry

_CACHED = {}
DEBUG = False


def _get_nc():
    if "nc" not in _CACHED:
        _CACHED["nc"] = _build_nc()
    return _CACHED["nc"]


PROFILE = False


def kernel(**inputs):
    in_maps, vid_all = _host_prep(inputs)
    nc = _get_nc()
    res = run_bass_kernel_spmd(nc, in_maps, core_ids=list(range(NCORES)),
                               trace=PROFILE)
    _CACHED["last_res"] = res
    out = np.zeros((B, VS, 3), np.float32)
    for c in range(NCORES):
        o = np.asarray(res.results[c]["out"]).reshape(ROWS, 3, B)
        vok = vid_all[c] >= 0
        out[:, vid_all[c][vok], :] = o[vok].transpose(2, 0, 1)
    return out


# revision 24
# speedup vs baseline: 1.1967x; 1.1967x over previous
"""EHM (SMPLX body + FLAME head + MANO hands) Bass kernel for 8 TRN2 NeuronCores.

Sharding: VERTEX sharding — each core owns 1/8 of the SMPLX vertices (plus the
FLAME/MANO vertices its SMPLX rows stitch in) and computes ALL B=128 batch
elements for its shard.

Key structure (v2):
  * Joint regression is linear in betas, so J = (J_reg @ [shapedirs|tmpl]) @
    [beta;1] is precomputed on the host into small `jsd` matrices — no
    AllReduce, no per-vertex J partials, and FK runs at the start of the
    kernel fully overlapped with blend-shape matmuls.
  * MANO betas are batch-constant: mano v_shaped / joints / rel-joints are
    host constants.
  * Head-stitched SMPLX rows only need their posedirs term (their blend value
    is replaced), so those chunks skip the shapedirs matmuls.
  * Shapedirs+posedirs accumulate in one PSUM group per chunk.
  * Stitch biases ride as extra rows of the skinning rhs (no bcast matmuls).
  * FK forest is split: SMPLX tree on the vector engine, FLAME+hands on
    gpsimd(Pool); skinning t_apply alternates between the two engines.

Per-vertex data layout: [vertex(partition<=128), (c, b)] with c-major free dim
(col = c*128 + b).  Batch-staged data (poses, FK, A matrices): [b(part), free].
"""

import sys

sys.path.insert(0, "/opt/trn_rl_repo")

from contextlib import ExitStack

import numpy as np
import ml_dtypes

BF16NP = ml_dtypes.bfloat16

import concourse.bass as bass
import concourse.bacc as bacc
import concourse.tile as tile
import concourse.mybir as mybir
from concourse.bass_utils import run_bass_kernel_spmd

F32 = mybir.dt.float32
BF16 = mybir.dt.bfloat16
AF = mybir.ActivationFunctionType
ALU = mybir.AluOpType

# ---------------------------------------------------------------- constants
B = 128
VS, VF, VM = 10475, 5023, 778
NL = 350
NCORES = 8

SMPLX_PARENTS = np.array([-1,0,0,0,1,2,3,4,5,6,7,8,9,9,9,12,13,14,16,17,18,19,
                          15,15,15,20,25,26,20,28,29,20,31,32,20,34,35,20,37,38,
                          21,40,41,21,43,44,21,46,47,21,49,50,21,52,53])
FLAME_PARENTS = np.array([-1,0,1,1,1])
MANO_PARENTS = np.array([-1,0,1,2,0,4,5,0,7,8,0,10,11,0,13,14])

N_PLAIN, N_HEAD, N_HL, N_HR = 768, 384, 128, 128
ROWS = N_PLAIN + N_HEAD + N_HL + N_HR        # 1408
NCH = ROWS // 128                            # 11
CH_PLAIN = set(range(0, 6))
CH_HEAD0 = 6                                 # chunks 6,7,8 head; 9 L; 10 R
CH_HL, CH_HR = 9, 10

PD_S_K = 189
PD_F_K = 27
PD_M_K = 135

NROT = 55
ROT_S0, ROT_F0, ROT_L0, ROT_R0 = 0, 22, 25, 40

# pool-forest joint offsets (flame 0..4, left hand 5..20, right hand 21..36)
PF_F, PF_L, PF_R = 0, 5, 21
NJ_P = 37

BF16_INPUTS = {"w_s", "wre_f", "w_m",
               "sd_s", "pd_s_a", "pd_s_b", "sd_f", "pd_f",
               "pd_m_a", "pd_m_b", "jsds", "jsdf",
               "betaT_s", "betaT_f"}


def _fk_levels(par):
    nj = len(par)
    depth = np.zeros(nj, np.int64)
    for j in range(nj):
        if par[j] >= 0:
            depth[j] = depth[par[j]] + 1
    levels = []
    for d in range(1, int(depth.max()) + 1):
        js = np.nonzero(depth == d)[0]
        runs, i = [], 0
        while i < len(js):
            j0, p0 = int(js[i]), int(par[js[i]])
            if i + 1 < len(js):
                ds = int(js[i + 1]) - j0
                ps = int(par[js[i + 1]]) - p0
            else:
                ds, ps = 1, 0
            n = 1
            while (i + n < len(js) and int(js[i + n]) == j0 + n * ds
                   and int(par[js[i + n]]) == p0 + n * ps):
                n += 1
            if n == 1:
                ds, ps = 1, 0
            runs.append((j0, ds, n, p0, ps))
            i += n
        levels.append(runs)
    return levels


def _par_pool():
    par = np.empty(NJ_P, np.int64)
    par[PF_F:PF_F + 5] = np.where(FLAME_PARENTS < 0, -1, FLAME_PARENTS + PF_F)
    par[PF_L:PF_L + 16] = np.where(MANO_PARENTS < 0, -1, MANO_PARENTS + PF_L)
    par[PF_R:PF_R + 16] = np.where(MANO_PARENTS < 0, -1, MANO_PARENTS + PF_R)
    return par


# ================================================================ host prep

def _split_sizes(total, parts):
    q, r = divmod(total, parts)
    return [q + (1 if i < r else 0) for i in range(parts)]


def _pad_ids(ids, n):
    out = np.full(n, -1, np.int64)
    out[:len(ids)] = ids
    return out


def _mrel(par, nj):
    m = np.eye(nj, dtype=np.float32)
    for j in range(1, nj):
        if par[j] >= 0:
            m[j, par[j]] = -1.0
    return m


def _host_prep(inp):
    f32 = np.float32
    s2f = np.asarray(inp["smplx2flame_ind"])
    head_ix = np.asarray(inp["head_index"])
    s2l = np.asarray(inp["smplx2mano_left"])
    s2r = np.asarray(inp["smplx2mano_right"])

    head_sv = s2f[head_ix]
    special = np.zeros(VS, bool)
    special[head_sv] = True
    special[s2l] = True
    special[s2r] = True
    plain_sv = np.nonzero(~special)[0]

    pl_sp = np.cumsum([0] + _split_sizes(len(plain_sv), NCORES))
    hd_sp = np.cumsum([0] + _split_sizes(len(head_ix), NCORES))
    hl_sp = np.cumsum([0] + _split_sizes(VM, NCORES))

    sd_s_np = np.asarray(inp["smplx_shapedirs"], f32)
    pd_s_np = np.asarray(inp["smplx_posedirs"], f32)
    jr_s_np = np.asarray(inp["smplx_J_regressor"], f32)
    w_s_np = np.asarray(inp["smplx_lbs_weights"], f32)
    tmpl_s = np.asarray(inp["smplx_v_template"], f32)
    sd_f_np = np.asarray(inp["flame_shapedirs"], f32)
    pd_f_np = np.asarray(inp["flame_posedirs"], f32)
    jr_f_np = np.asarray(inp["flame_J_regressor"], f32)
    w_f_np = np.asarray(inp["flame_lbs_weights"], f32)
    tmpl_f = np.asarray(inp["flame_v_template"], f32)
    re_np = np.asarray(inp["r_eyelid"], f32)
    le_np = np.asarray(inp["l_eyelid"], f32)
    sd_m_np = np.asarray(inp["mano_shapedirs"], f32)
    pd_m_np = np.asarray(inp["mano_posedirs"], f32)
    jr_m_np = np.asarray(inp["mano_J_regressor"], f32)
    w_m_np = np.asarray(inp["mano_lbs_weights"], f32)
    tmpl_m = np.asarray(inp["mano_v_template"], f32)

    aa = np.concatenate([
        np.asarray(inp["global_pose"], f32).reshape(B, 3),
        np.asarray(inp["body_pose"], f32).reshape(B, 63),
        np.asarray(inp["jaw_params"], f32).reshape(B, 3),
        np.asarray(inp["eye_pose"], f32).reshape(B, 6),
        np.asarray(inp["left_hand_pose"], f32).reshape(B, 45),
        np.asarray(inp["right_hand_pose"], f32).reshape(B, 45),
    ], axis=1)

    ep = np.asarray(inp["eyelid_params"], f32)
    aux = np.concatenate([
        np.asarray(inp["head_scale"], f32)[:, None],
        np.asarray(inp["left_hand_scale"], f32)[:, None],
        np.asarray(inp["right_hand_scale"], f32)[:, None],
        ep[:, 0:1], ep[:, 1:2],
        np.asarray(inp["head_pos_offset"], f32),
        np.asarray(inp["left_hand_pos_offset"], f32),
        np.asarray(inp["right_hand_pos_offset"], f32),
    ], axis=1)                                               # [128, 14]

    def beta_T(second):
        b = np.concatenate([np.asarray(inp["shape_params"], f32), second], 1)
        bt = np.zeros((384, B), f32)
        bt[:NL] = b.T
        bt[NL] = 1.0
        return bt.reshape(3, 128, B)

    betaT_s = beta_T(np.asarray(inp["body_exp"], f32))
    betaT_f = beta_T(np.asarray(inp["flame_exp"], f32))

    mrel_s = _mrel(SMPLX_PARENTS, 55)
    mrel_f = _mrel(FLAME_PARENTS, 5)
    mrel_m = _mrel(MANO_PARENTS, 16)

    def jsd_slab(jr, sd, tmpl, mrel, nj):
        jsd = np.einsum('jv,vcl->lcj', jr, sd)           # [350, 3, nj]
        jt = jr @ tmpl                                    # [nj, 3]
        jsd_r = np.einsum('lcj,kj->lck', jsd, mrel)
        jt_r = mrel @ jt
        slab = np.zeros((384, 2 * 3 * nj), f32)
        slab[:NL, :3 * nj] = jsd.reshape(NL, 3 * nj)
        slab[NL, :3 * nj] = jt.T.reshape(3 * nj)
        slab[:NL, 3 * nj:] = jsd_r.reshape(NL, 3 * nj)
        slab[NL, 3 * nj:] = jt_r.T.reshape(3 * nj)
        return np.ascontiguousarray(slab.reshape(3, 128, 2 * 3 * nj))

    jsds = jsd_slab(jr_s_np, sd_s_np, tmpl_s, mrel_s, 55)
    jsdf = jsd_slab(jr_f_np, sd_f_np, tmpl_f, mrel_f, 5)

    joff = np.asarray(inp["joints_offset"], f32)          # [B, 55, 3]
    joffB = np.ascontiguousarray(joff.transpose(0, 2, 1).reshape(B, 165))
    joffrelB = np.ascontiguousarray(
        np.einsum('kj,bjc->bck', mrel_s, joff).reshape(B, 165))

    mb = np.asarray(inp["mano_betas"], f32)[0]
    vsh_m = tmpl_m + sd_m_np @ mb                          # [778, 3]
    J_m = jr_m_np @ vsh_m
    rel_m = mrel_m @ J_m
    jmb_c = np.ascontiguousarray(np.broadcast_to(J_m.T.reshape(1, 48), (B, 48)))
    relmb_c = np.ascontiguousarray(np.broadcast_to(rel_m.T.reshape(1, 48), (B, 48)))

    rep = dict(aa=aa, aux=aux, betaT_s=betaT_s, betaT_f=betaT_f,
               joffB=joffB, joffrelB=joffrelB, jmb_c=jmb_c, relmb_c=relmb_c,
               jsds=jsds, jsdf=jsdf, ident=np.eye(128, dtype=f32))

    in_maps = []
    vid_all = np.full((NCORES, ROWS), -1, np.int64)

    for c in range(NCORES):
        p_ids = plain_sv[pl_sp[c]:pl_sp[c + 1]]
        h_pos = np.arange(hd_sp[c], hd_sp[c + 1])
        h_sv, h_fv = head_sv[h_pos], head_ix[h_pos]
        l_pos = np.arange(hl_sp[c], hl_sp[c + 1])
        l_sv, r_sv = s2l[l_pos], s2r[l_pos]

        vid = np.full(ROWS, -1, np.int64)
        vid[:len(p_ids)] = p_ids
        vid[N_PLAIN:N_PLAIN + len(h_sv)] = h_sv
        vid[N_PLAIN + N_HEAD:N_PLAIN + N_HEAD + len(l_sv)] = l_sv
        vid[N_PLAIN + N_HEAD + N_HL:N_PLAIN + N_HEAD + N_HL + len(r_sv)] = r_sv
        vid_all[c] = vid
        vok = vid >= 0
        vc = np.where(vok, vid, 0)

        # smplx shapedirs slab, PLAIN chunks only: [6, 128(l), (c, lk, v)]
        pok, pc = vok[:N_PLAIN], vc[:N_PLAIN]
        sdp = np.zeros((N_PLAIN, 3, 384), f32)
        sdp[:, :, :NL] = np.where(pok[:, None, None], sd_s_np[pc], 0.0)
        sdp[:, :, NL] = np.where(pok[:, None], tmpl_s[pc], 0.0)
        slab = sdp.reshape(6, 128, 3, 3, 128).transpose(0, 4, 2, 3, 1)
        sd_s = np.ascontiguousarray(slab).reshape(6, 128, 1152)

        colv = vc[:, None] * 3 + np.arange(3)[None, :]
        pdv = pd_s_np[:PD_S_K][:, colv]
        pdv = np.where(vok[None, :, None], pdv, 0.0)
        pdv = pdv.reshape(PD_S_K, NCH, 128, 3).transpose(1, 0, 3, 2)
        pd_s_a = np.ascontiguousarray(pdv[:, :128]).reshape(NCH, 128, 384)
        pd_s_b = np.ascontiguousarray(pdv[:, 128:]).reshape(NCH, PD_S_K - 128, 384)

        w_s = np.ascontiguousarray(
            np.where(vok[:, None], w_s_np[vc], 0.0)
            .reshape(NCH, 128, 55).transpose(0, 2, 1))

        # flame gathered chunks
        fg = _pad_ids(h_fv, N_HEAD)
        fok = fg >= 0
        fc = np.where(fok, fg, 0)
        sdfp = np.zeros((N_HEAD, 3, 384), f32)
        sdfp[:, :, :NL] = np.where(fok[:, None, None], sd_f_np[fc], 0.0)
        sdfp[:, :, NL] = np.where(fok[:, None], tmpl_f[fc], 0.0)
        slab = sdfp.reshape(3, 128, 3, 3, 128).transpose(0, 4, 2, 3, 1)
        sd_f = np.ascontiguousarray(slab).reshape(3, 128, 1152)

        colf = fc[:, None] * 3 + np.arange(3)[None, :]
        pdfv = pd_f_np[9:36][:, colf]
        pdfv = np.where(fok[None, :, None], pdfv, 0.0)
        pdfv = pdfv.reshape(PD_F_K, 3, 128, 3).transpose(1, 0, 3, 2)
        pd_f = np.ascontiguousarray(pdfv).reshape(3, PD_F_K, 384)

        wre = np.zeros((3, 12, 128), f32)
        for k in range(3):
            rows, ok = fc[k * 128:(k + 1) * 128], fok[k * 128:(k + 1) * 128]
            wre[k, :5] = np.where(ok[None, :], w_f_np[rows].T, 0.0)
            wre[k, 5:8] = np.where(ok[None, :], re_np[rows].T, 0.0)
            wre[k, 8:11] = np.where(ok[None, :], le_np[rows].T, 0.0)
            wre[k, 11] = np.where(ok, 1.0, 0.0)

        # mano (left/right share vertex ids -> shared pd, w, vshm)
        m_rows = _pad_ids(l_pos, 128)
        mok = m_rows >= 0
        mc = np.where(mok, m_rows, 0)
        colm = mc[:, None] * 3 + np.arange(3)[None, :]
        pdm = pd_m_np[:, colm]
        pdm = np.where(mok[None, :, None], pdm, 0.0).transpose(0, 2, 1)
        pd_m_a = np.ascontiguousarray(pdm[:128].reshape(128, 384))
        pd_m_b = np.ascontiguousarray(pdm[128:].reshape(PD_M_K - 128, 384))
        w_m = np.zeros((17, 128), f32)
        w_m[:16] = np.where(mok[None, :], w_m_np[mc].T, 0.0)
        w_m[16] = np.where(mok, 1.0, 0.0)
        vshm = np.where(mok[:, None], vsh_m[mc], 0.0).astype(f32)  # [128, 3]

        m = dict(rep)
        m.update(sd_s=sd_s, pd_s_a=pd_s_a, pd_s_b=pd_s_b, w_s=w_s,
                 sd_f=sd_f, pd_f=pd_f, wre_f=wre,
                 pd_m_a=pd_m_a, pd_m_b=pd_m_b, w_m=w_m, vshm=vshm)
        out = {}
        for k, v in m.items():
            if k in BF16_INPUTS:
                out[k] = np.ascontiguousarray(v.astype(BF16NP))
            else:
                out[k] = np.ascontiguousarray(v, f32)
        in_maps.append(out)

    return in_maps, vid_all


# ================================================================ device IR

def _build_nc():
    nc = bacc.Bacc("TRN2", target_bir_lowering=False, debug=False,
                   num_devices=NCORES)
    di = {}

    def din(name, shape):
        dt = BF16 if name in BF16_INPUTS else F32
        di[name] = nc.dram_tensor(name, list(shape), dt, kind="ExternalInput").ap()

    din("aa", (B, 165)); din("aux", (B, 14))
    din("betaT_s", (3, 128, 128)); din("betaT_f", (3, 128, 128))
    din("joffB", (B, 165)); din("joffrelB", (B, 165))
    din("jmb_c", (B, 48)); din("relmb_c", (B, 48))
    din("jsds", (3, 128, 330)); din("jsdf", (3, 128, 30))
    din("ident", (128, 128))
    din("sd_s", (6, 128, 1152))
    din("pd_s_a", (NCH, 128, 384)); din("pd_s_b", (NCH, PD_S_K - 128, 384))
    din("w_s", (NCH, 55, 128))
    din("sd_f", (3, 128, 1152)); din("pd_f", (3, PD_F_K, 384))
    din("wre_f", (3, 12, 128))
    din("pd_m_a", (128, 384)); din("pd_m_b", (PD_M_K - 128, 384))
    din("w_m", (17, 128)); din("vshm", (128, 3))

    out_d = nc.dram_tensor("out", [ROWS, 384], F32, kind="ExternalOutput").ap()
    dbg_d = None
    if DEBUG:
        dbg_d = nc.dram_tensor("dbg", [128, 4096], F32, kind="ExternalOutput").ap()

    with tile.TileContext(nc) as tc:
        _emit(nc, tc, di, out_d, dbg_d)
    nc.compile()
    return nc


def _emit(nc, tc, di, out_d, dbg_d=None):
    par_p = _par_pool()
    levels_s = _fk_levels(SMPLX_PARENTS)
    levels_p = _fk_levels(par_p)
    es = ExitStack()
    persist = es.enter_context(tc.tile_pool(name="persist", bufs=1))
    slabs = es.enter_context(tc.tile_pool(name="slabs", bufs=3))
    acc_cm = tc.tile_pool(name="acc", bufs=2, space="PSUM")
    acc = acc_cm.__enter__()
    tpz_cm = tc.tile_pool(name="tpz", bufs=2, space="PSUM")
    tpz = tpz_cm.__enter__()

    V, S, G, T, DMA = nc.vector, nc.scalar, nc.gpsimd, nc.tensor, nc.sync

    def ptile(shape, name, dt=F32):
        return persist.tile(list(shape), dt, tag=name, name=name)

    # ---------------- staged inputs (latency-critical first) --------------
    aa = ptile((B, 165), "aa"); DMA.dma_start(aa[:], di["aa"][:])
    aux = ptile((B, 14), "aux"); DMA.dma_start(aux[:], di["aux"][:])
    betaT_s = ptile((128, 384), "betaT_s", BF16)
    betaT_f = ptile((128, 384), "betaT_f", BF16)
    jsds = ptile((128, 3 * 330), "jsds", BF16)
    jsdf = ptile((128, 3 * 30), "jsdf", BF16)
    for lk in range(3):
        DMA.dma_start(betaT_s[:, lk * 128:(lk + 1) * 128], di["betaT_s"][lk])
        DMA.dma_start(jsds[:, lk * 330:(lk + 1) * 330], di["jsds"][lk])
    for lk in range(3):
        DMA.dma_start(betaT_f[:, lk * 128:(lk + 1) * 128], di["betaT_f"][lk])
        DMA.dma_start(jsdf[:, lk * 30:(lk + 1) * 30], di["jsdf"][lk])
    joffB = ptile((B, 165), "joffB"); DMA.dma_start(joffB[:], di["joffB"][:])
    joffrelB = ptile((B, 165), "joffrelB")
    DMA.dma_start(joffrelB[:], di["joffrelB"][:])
    ident = ptile((128, 128), "ident")
    DMA.dma_start(ident[:], di["ident"][:])
    jmb = ptile((B, 48), "jmb"); DMA.dma_start(jmb[:], di["jmb_c"][:])
    relmb = ptile((B, 48), "relmb"); DMA.dma_start(relmb[:], di["relmb_c"][:])
    vshm = ptile((128, 3), "vshm"); DMA.dma_start(vshm[:], di["vshm"][:])

    # warm the scalar-engine activation tables before real work needs them
    warm = ptile((B, 1), "warm")
    warm2 = ptile((B, 1), "warm2")
    nc.gpsimd.memset(warm[:], 0.25)
    S.activation(warm2[:], warm[:], AF.Sqrt, bias=warm[:])
    S.activation(warm2[:], warm[:], AF.Sin, bias=warm[:])

    # ---------------- rodrigues ------------------------------------------
    rot = ptile((B, NROT * 9), "rot")
    _rodrigues(nc, aa, rot, ptile)
    rot4 = rot[:].rearrange("p (j x) -> p j x", x=9)

    def pf_make(name, j0, n):
        t = ptile((B, n * 9), name)
        t9 = t[:].rearrange("p (j x) -> p j x", x=9)
        V.tensor_copy(t9, rot4[:, j0:j0 + n, :])
        V.tensor_scalar_add(t9[:, :, 0:9:4], t9[:, :, 0:9:4], -1.0)
        return t

    pf_s = pf_make("pf_s", 1, 21)
    pf_f = pf_make("pf_f", 22, 3)
    pf_m = [pf_make("pf_l", 25, 15), pf_make("pf_r", 40, 15)]
    epp = ptile((B, 2), "epp")
    V.tensor_mul(epp[:], aux[:, 3:5], aux[:, 0:1].broadcast_to([B, 2]))

    def transpose_to(dst_ap, src_ap):
        pp = tpz.tile([128, 512], F32, tag="tpose")
        k, n = src_ap.shape[0], src_ap.shape[1]
        T.matmul(pp[:n, :k], src_ap, ident[:k, :k], is_transpose=True,
                 start=True, stop=True)
        S.copy(dst_ap, pp[:n, :k])

    pfT_s_a = ptile((128, 128), "pfT_s_a", BF16)
    pfT_s_b = ptile((PD_S_K - 128, 128), "pfT_s_b", BF16)
    transpose_to(pfT_s_a[:], pf_s[:, 0:128])
    transpose_to(pfT_s_b[:], pf_s[:, 128:PD_S_K])
    pfT_f = ptile((PD_F_K, 128), "pfT_f", BF16)
    transpose_to(pfT_f[:], pf_f[:, :])
    pfT_m_a = [ptile((128, 128), "pfT_l_a", BF16), ptile((128, 128), "pfT_r_a", BF16)]
    pfT_m_b = [ptile((PD_M_K - 128, 128), "pfT_l_b", BF16),
               ptile((PD_M_K - 128, 128), "pfT_r_b", BF16)]
    for h in range(2):
        transpose_to(pfT_m_a[h][:], pf_m[h][:, 0:128])
        transpose_to(pfT_m_b[h][:], pf_m[h][:, 128:PD_M_K])
    epT = ptile((2, 128), "epT", BF16)
    transpose_to(epT[:], epp[:, :])

    # ---------------- J from betas (host-precomputed jsd) -----------------
    jsp = acc.tile([128, 512], F32, tag="vppsum", name="jsp")
    for lk in range(3):
        T.matmul(jsp[:, 0:330], betaT_s[:, lk * 128:(lk + 1) * 128],
                 jsds[:, lk * 330:(lk + 1) * 330],
                 start=(lk == 0), stop=(lk == 2))
    jfp = acc.tile([128, 512], F32, tag="vppsum", name="jfp")
    for lk in range(3):
        T.matmul(jfp[:, 0:30], betaT_f[:, lk * 128:(lk + 1) * 128],
                 jsdf[:, lk * 30:(lk + 1) * 30],
                 start=(lk == 0), stop=(lk == 2))

    jb = ptile((B, 165), "jb")
    relb = ptile((B, 165), "relb")
    V.tensor_add(jb[:], jsp[:, 0:165], joffB[:])
    V.tensor_add(relb[:], jsp[:, 165:330], joffrelB[:])
    jfb = ptile((B, 15), "jfb")
    relfb = ptile((B, 15), "relfb")
    S.copy(jfb[:], jfp[:, 0:15])
    S.copy(relfb[:], jfp[:, 15:30])

    # ---------------- FK: smplx on vector, flame+hands on pool ------------
    Tb_s = ptile((B, 55 * 12), "Tb_s")
    Ab_s = ptile((B, 55 * 12), "Ab_s")
    T4s = Tb_s[:].rearrange("p (j m n) -> p j m n", m=3, n=4)
    A4s = Ab_s[:].rearrange("p (j m n) -> p j m n", m=3, n=4)
    Tb_p = ptile((B, NJ_P * 12), "Tb_p")
    Ab_p = ptile((B, NJ_P * 12), "Ab_p")
    T4p = Tb_p[:].rearrange("p (j m n) -> p j m n", m=3, n=4)
    A4p = Ab_p[:].rearrange("p (j m n) -> p j m n", m=3, n=4)

    V.memset(Tb_s[:], 0.0)
    G.memset(Tb_p[:], 0.0)
    V.tensor_copy(T4s[:, 0:22, :, 0:3],
                  rot4[:, 0:22, :].rearrange("p j (m n) -> p j m n", n=3))
    V.memset(Tb_s[:].rearrange("p (j x) -> p j x", x=12)[:, 22:55, 0:11:5], 1.0)
    V.tensor_copy(T4s[:, 0:55, :, 3], relb[:].rearrange("p (c j) -> p j c", c=3))

    G.tensor_copy(T4p[:, PF_F + 2:PF_F + 5, :, 0:3],
                  rot4[:, ROT_F0:ROT_F0 + 3, :].rearrange("p j (m n) -> p j m n", n=3))
    G.tensor_copy(T4p[:, PF_L + 1:PF_L + 16, :, 0:3],
                  rot4[:, ROT_L0:ROT_L0 + 15, :].rearrange("p j (m n) -> p j m n", n=3))
    G.tensor_copy(T4p[:, PF_R + 1:PF_R + 16, :, 0:3],
                  rot4[:, ROT_R0:ROT_R0 + 15, :].rearrange("p j (m n) -> p j m n", n=3))
    for j0, n in ((PF_F, 2), (PF_L, 1), (PF_R, 1)):
        G.memset(Tb_p[:].rearrange("p (j x) -> p j x", x=12)[:, j0:j0 + n, 0:11:5], 1.0)
    G.tensor_copy(T4p[:, PF_F:PF_F + 5, :, 3],
                  relfb[:].rearrange("p (c j) -> p j c", c=3))
    for off in (PF_L, PF_R):
        G.tensor_copy(T4p[:, off:off + 16, :, 3],
                      relmb[:].rearrange("p (c j) -> p j c", c=3))

    def fk_forest(E, T4, A4, levels, roots, tmp_t, tmp2_t):
        for r in roots:
            E.tensor_copy(A4[:, r:r + 1], T4[:, r:r + 1])
        for runs in levels:
            for (d0, ds, n, p0, ps) in runs:
                sl_d = slice(d0, d0 + (n - 1) * ds + 1, ds) if ds != 1 else slice(d0, d0 + n)
                dst, dT = A4[:, sl_d], T4[:, sl_d]
                if ps == 0:
                    par = A4[:, p0:p0 + 1].broadcast_to([B, n, 3, 4])
                else:
                    sl_p = slice(p0, p0 + (n - 1) * ps + 1, ps) if ps != 1 else slice(p0, p0 + n)
                    par = A4[:, sl_p]
                sc2 = tmp2_t[:].rearrange("p (j m n) -> p j m n", m=3, n=4)[:, :n]
                # children and parents are disjoint joint ranges: accumulate
                # straight into A4 (no bounce copy)
                for k in range(3):
                    a_k = par[:, :, :, k:k + 1].broadcast_to([B, n, 3, 4])
                    t_k = dT[:, :, k:k + 1, :].broadcast_to([B, n, 3, 4])
                    if k == 0:
                        E.tensor_mul(dst, a_k, t_k)
                    else:
                        E.tensor_mul(sc2, a_k, t_k)
                        E.tensor_add(dst, dst, sc2)
                E.tensor_add(dst[:, :, :, 3], dst[:, :, :, 3], par[:, :, :, 3])

    fk_tmp_s = ptile((B, 12 * 16), "fk_tmp_s")
    fk_tmp2_s = ptile((B, 12 * 16), "fk_tmp2_s")
    fk_tmp_p = ptile((B, 12 * 16), "fk_tmp_p")
    fk_tmp2_p = ptile((B, 12 * 16), "fk_tmp2_p")
    fk_forest(V, T4s, A4s, levels_s, (0,), fk_tmp_s, fk_tmp2_s)
    fk_forest(G, T4p, A4p, levels_p, (PF_F, PF_L, PF_R), fk_tmp_p, fk_tmp2_p)

    # ---- stitch biases (world translations BEFORE rel-correction) --------
    hm = ptile((B, 16), "hm")
    jb3 = jb[:].rearrange("p (c j) -> p c j", c=3)
    jm3 = jmb[:].rearrange("p (c j) -> p c j", c=3)
    bias9 = ptile((B, 9), "bias9")
    V.tensor_add(hm[:, 0:3], jb3[:, :, 23], jb3[:, :, 24])
    V.tensor_add(hm[:, 3:6], A4p[:, PF_F + 3, :, 3], A4p[:, PF_F + 4, :, 3])
    V.tensor_sub(hm[:, 6:9], hm[:, 0:3], hm[:, 3:6])
    V.tensor_scalar_mul(hm[:, 6:9], hm[:, 6:9], 0.5)
    V.tensor_add(bias9[:, 0:3], hm[:, 6:9], aux[:, 5:8])
    V.tensor_sub(hm[:, 9:12], aux[:, 8:11], jm3[:, :, 0])
    V.tensor_sub(bias9[:, 3:4], jb3[:, 0:1, 20], hm[:, 9:10])
    V.tensor_add(bias9[:, 4:6], hm[:, 10:12], jb3[:, 1:3, 20])
    V.tensor_sub(hm[:, 12:15], aux[:, 11:14], jm3[:, :, 0])
    V.tensor_add(bias9[:, 6:9], hm[:, 12:15], jb3[:, :, 21])

    # ---- A_rel: translation -= R_world @ J ------------------------------
    corr_s = ptile((B, 55 * 3), "corr_s")
    corr_s2 = ptile((B, 55 * 3), "corr_s2")
    corr_p = ptile((B, 16 * 3), "corr_p")
    corr_p2 = ptile((B, 16 * 3), "corr_p2")

    def corr(E, A4, j0, nj, jsrc, ct_t, ct2_t):
        ct = ct_t[:].rearrange("p (j m) -> p j m", m=3)[:, 0:nj]
        ct2 = ct2_t[:].rearrange("p (j m) -> p j m", m=3)[:, 0:nj]
        js = jsrc.rearrange("p (c j) -> p c j", c=3)
        for k in range(3):
            a_k = A4[:, j0:j0 + nj, :, k]
            j_k = js[:, k, :].unsqueeze(2).broadcast_to([B, nj, 3])
            if k == 0:
                E.tensor_mul(ct, a_k, j_k)
            else:
                E.tensor_mul(ct2, a_k, j_k)
                E.tensor_add(ct, ct, ct2)
        E.tensor_sub(A4[:, j0:j0 + nj, :, 3], A4[:, j0:j0 + nj, :, 3], ct)

    corr(V, A4s, 0, 55, jb[:], corr_s, corr_s2)
    corr(G, A4p, PF_F, 5, jfb[:], corr_p, corr_p2)
    corr(G, A4p, PF_L, 16, jmb[:], corr_p, corr_p2)
    corr(G, A4p, PF_R, 16, jmb[:], corr_p, corr_p2)

    # ---- scale folding (pool tile) --------------------------------------
    G.tensor_scalar_mul(Ab_p[:, PF_F * 12:(PF_F + 5) * 12],
                        Ab_p[:, PF_F * 12:(PF_F + 5) * 12], aux[:, 0:1])
    negls = ptile((B, 1), "negls")
    G.tensor_scalar_mul(negls[:], aux[:, 1:2], -1.0)
    AL = A4p[:, PF_L:PF_L + 16]
    G.tensor_scalar_mul(AL[:, :, 0, :], AL[:, :, 0, :], negls[:, 0:1])
    G.tensor_scalar_mul(AL[:, :, 1:3, :], AL[:, :, 1:3, :], aux[:, 1:2])
    ARr = A4p[:, PF_R:PF_R + 16]
    G.tensor_scalar_mul(ARr[:, :, :, :], ARr[:, :, :, :], aux[:, 2:3])

    # ================= blend chunks (tensor) ==============================
    vp_sbuf = [ptile((128, 384), f"vp{i}", BF16) for i in range(NCH)]
    vpf_sbuf = [ptile((128, 384), f"vpf{h}", BF16) for h in range(3)]
    vpm_sbuf = [ptile((128, 384), f"vpm{h}", BF16) for h in range(2)]

    def blend_smplx(i):
        pda = slabs.tile((128, 384), BF16, tag="pd_s_a")
        pdb = slabs.tile((PD_S_K - 128, 384), BF16, tag="pd_s_b")
        DMA.dma_start(pda[:], di["pd_s_a"][i])
        DMA.dma_start(pdb[:], di["pd_s_b"][i])
        sdt = None
        if i in CH_PLAIN:
            sdt = slabs.tile((128, 1152), BF16, tag="sd_s")
            DMA.dma_start(sdt[:], di["sd_s"][i])
        pp = acc.tile([128, 512], F32, tag="vppsum")
        # one accumulation group per c3 (interleaved groups within a PSUM
        # bank corrupt accumulation on HW)
        for c3 in range(3):
            first = True
            if sdt is not None:
                for lk in range(3):
                    T.matmul(pp[:, c3 * 128:(c3 + 1) * 128],
                             sdt[:, (c3 * 3 + lk) * 128:(c3 * 3 + lk + 1) * 128],
                             betaT_s[:, lk * 128:(lk + 1) * 128],
                             start=first, stop=False)
                    first = False
            T.matmul(pp[:, c3 * 128:(c3 + 1) * 128],
                     pda[:, c3 * 128:(c3 + 1) * 128], pfT_s_a[:],
                     start=first, stop=False)
            T.matmul(pp[:, c3 * 128:(c3 + 1) * 128],
                     pdb[:, c3 * 128:(c3 + 1) * 128], pfT_s_b[:],
                     start=False, stop=True)
        S.copy(vp_sbuf[i][:], pp[:, 0:384])

    def blend_flame(h):
        sdt = slabs.tile((128, 1152), BF16, tag="sd_f")
        DMA.dma_start(sdt[:], di["sd_f"][h])
        pdf = slabs.tile((PD_F_K, 384), BF16, tag="pd_f")
        DMA.dma_start(pdf[:], di["pd_f"][h])
        pp = acc.tile([128, 512], F32, tag="vppsum")
        for c3 in range(3):
            for lk in range(3):
                T.matmul(pp[:, c3 * 128:(c3 + 1) * 128],
                         sdt[:, (c3 * 3 + lk) * 128:(c3 * 3 + lk + 1) * 128],
                         betaT_f[:, lk * 128:(lk + 1) * 128],
                         start=(lk == 0), stop=False)
            T.matmul(pp[:, c3 * 128:(c3 + 1) * 128],
                     pdf[:, c3 * 128:(c3 + 1) * 128], pfT_f[:],
                     start=False, stop=True)
        S.copy(vpf_sbuf[h][:], pp[:, 0:384])

    pdm_a = ptile((128, 384), "pdm_a", BF16)
    pdm_b = ptile((PD_M_K - 128, 384), "pdm_b", BF16)
    DMA.dma_start(pdm_a[:], di["pd_m_a"][:])
    DMA.dma_start(pdm_b[:], di["pd_m_b"][:])

    def blend_mano(h):
        pp = acc.tile([128, 512], F32, tag="vppsum")
        for c3 in range(3):
            T.matmul(pp[:, c3 * 128:(c3 + 1) * 128],
                     pdm_a[:, c3 * 128:(c3 + 1) * 128], pfT_m_a[h][:],
                     start=True, stop=False)
            T.matmul(pp[:, c3 * 128:(c3 + 1) * 128],
                     pdm_b[:, c3 * 128:(c3 + 1) * 128], pfT_m_b[h][:],
                     start=False, stop=True)
        vpm = vpm_sbuf[h]
        V.tensor_add(vpm[:].rearrange("p (c b) -> p c b", b=128),
                     pp[:, 0:384].rearrange("p (c b) -> p c b", b=128),
                     vshm[:].unsqueeze(2).broadcast_to([128, 3, 128]))

    for i in range(5):
        blend_smplx(i)

    # ---- rhs assembly (A matrices are ready by now) ----------------------
    def rhs_fill(rhs_t, A4, j0, nj, col0, n4):
        pp = tpz.tile([128, 512], F32, tag="tpose")
        for m3 in range(3):
            T.matmul(pp[0:nj, m3 * 128:(m3 + 1) * 128],
                     A4[:, j0:j0 + nj, m3, n4], ident[:],
                     is_transpose=True, start=True, stop=True)
        S.copy(rhs_t[0:nj, col0:col0 + 384], pp[0:nj, 0:384])

    rhs_s = persist.tile([55, 1536], BF16, tag="rhs_s", name="rhs_s")
    for n4 in range(4):
        rhs_fill(rhs_s, A4s, 0, 55, n4 * 384, n4)

    # eyelid and stitch-bias terms are purely additive -> they ride as extra
    # rows of the n=3 (translation) column group of the skinning rhs.
    rhs_f = persist.tile([12, 1536], BF16, tag="rhs_f", name="rhs_f")
    G.memset(rhs_f[:], 0.0)
    for n4 in range(4):
        rhs_fill(rhs_f, A4p, PF_F, 5, n4 * 384, n4)

    bias9T = ptile((9, 128), "bias9T", BF16)
    transpose_to(bias9T[:], bias9[:, :])
    for m3 in range(3):
        DMA.dma_start(rhs_f[5 + m3:6 + m3, 1152 + m3 * 128:1280 + m3 * 128],
                      epT[1:2, :])
        DMA.dma_start(rhs_f[8 + m3:9 + m3, 1152 + m3 * 128:1280 + m3 * 128],
                      epT[0:1, :])
        DMA.dma_start(rhs_f[11:12, 1152 + m3 * 128:1280 + m3 * 128],
                      bias9T[m3:m3 + 1, :])

    rhs_m = [persist.tile([17, 1536], BF16, tag="rhs_l", name="rhs_l"),
             persist.tile([17, 1536], BF16, tag="rhs_r", name="rhs_r")]
    for h, off in ((0, PF_L), (1, PF_R)):
        G.memset(rhs_m[h][:], 0.0)
        for n4 in range(4):
            rhs_fill(rhs_m[h], A4p, off, 16, n4 * 384, n4)
        for m3 in range(3):
            DMA.dma_start(rhs_m[h][16:17, 1152 + m3 * 128:1280 + m3 * 128],
                          bias9T[3 * (h + 1) + m3:3 * (h + 1) + m3 + 1, :])

    if dbg_d is not None:
        DMA.dma_start(dbg_d[0:12, 0:1536], rhs_f[:])
        DMA.dma_start(dbg_d[16:33, 0:1536], rhs_m[0][:])
        DMA.dma_start(dbg_d[40:95, 0:1536], rhs_s[:])
        DMA.dma_start(dbg_d[100:101, 0:165], jb[0:1, :])
        DMA.dma_start(dbg_d[101:102, 0:165], relb[0:1, :])
        DMA.dma_start(dbg_d[102:103, 0:15], jfb[0:1, :])

    # ================= skinning, interleaved with remaining blends ========
    tpz_cm.__exit__(None, None, None)
    big_cm = tc.tile_pool(name="big", bufs=2, space="PSUM")
    big = big_cm.__enter__()

    def t_apply(E, dst_ap, tpsb_ap, x_sbuf, s0, s1):
        """dst = sum_{n<3} T'[n]*x_n + T'[3]; all-bf16 ops (DVE 2x mode),
        f32 conversion only at the final add."""
        x3 = x_sbuf[:].rearrange("p (c b) -> p c b", b=128)
        tp = tpsb_ap.rearrange("p (n m b) -> p n m b", m=3, b=128)
        a0 = s0.rearrange("p (m b) -> p m b", b=128)
        a1 = s1.rearrange("p (m b) -> p m b", b=128)
        d3 = dst_ap.rearrange("p (m b) -> p m b", b=128)
        E.tensor_mul(a0, tp[:, 0], x3[:, 0:1].broadcast_to([128, 3, 128]))
        E.tensor_mul(a1, tp[:, 1], x3[:, 1:2].broadcast_to([128, 3, 128]))
        E.tensor_add(a0, a0, a1)
        E.tensor_mul(a1, tp[:, 2], x3[:, 2:3].broadcast_to([128, 3, 128]))
        E.tensor_add(a0, a0, a1)
        E.tensor_add(d3, a0, tp[:, 3])

    scr_v = [ptile((128, 384), f"scrv{i}", BF16) for i in range(4)]
    scr_g = [ptile((128, 384), f"scrg{i}", BF16) for i in range(4)]
    cnt_v, cnt_g = [0], [0]

    def skin_mm(wt, rhs_t):
        tp = big.tile([128, 1536], F32, tag="bigp")
        for g in range(3):
            T.matmul(tp[:, g * 512:(g + 1) * 512], wt[:],
                     rhs_t[:, g * 512:(g + 1) * 512], start=True, stop=True)
        return tp

    def apply_eng(E, dst_ap, tp_psum, x_sbuf):
        tpsb = slabs.tile((128, 1536), BF16, tag="tpsb", bufs=3, name="tpsb")
        S.copy(tpsb[:], tp_psum[:])
        if E is G:
            cnt_g[0] += 1
            k = (cnt_g[0] * 2) % 4
            t_apply(G, dst_ap, tpsb[:], x_sbuf, scr_g[k][:], scr_g[k + 1][:])
        else:
            cnt_v[0] += 1
            k = (cnt_v[0] * 2) % 4
            t_apply(V, dst_ap, tpsb[:], x_sbuf, scr_v[k][:], scr_v[k + 1][:])

    def skin_chunk(i):
        if CH_HEAD0 <= i < CH_HEAD0 + 3:
            h = i - CH_HEAD0
            hv = slabs.tile((128, 384), BF16, tag="hv", bufs=2, name="hv")
            wt = slabs.tile((12, 128), BF16, tag="wre_f")
            DMA.dma_start(wt[:], di["wre_f"][h])
            tp1 = skin_mm(wt, rhs_f)
            apply_eng(G, hv[:], tp1, vpf_sbuf[h])
            G.tensor_add(vp_sbuf[i][:], vp_sbuf[i][:], hv[:])
        elif i in (CH_HL, CH_HR):
            h = i - CH_HL
            hv = slabs.tile((128, 384), BF16, tag="hv", bufs=2, name="hv")
            wt = slabs.tile((17, 128), BF16, tag="w_m")
            DMA.dma_start(wt[:], di["w_m"][:])
            tpm = skin_mm(wt, rhs_m[h])
            apply_eng(G, hv[:], tpm, vpm_sbuf[h])
            G.tensor_add(vp_sbuf[i][:], vp_sbuf[i][:], hv[:])

        wt = slabs.tile((55, 128), BF16, tag="w_s")
        DMA.dma_start(wt[:], di["w_s"][i])
        tps = skin_mm(wt, rhs_s)
        ot = slabs.tile((128, 384), F32, tag="outt", bufs=3, name="ot")
        apply_eng(G if (i in CH_PLAIN and i % 2 == 1) else V,
                  ot[:], tps, vp_sbuf[i])
        DMA.dma_start(out_d[i * 128:(i + 1) * 128, :], ot[:])

    for i in range(6):
        skin_chunk(i)
        blend_smplx(i + 5)
    for h in range(3):
        blend_flame(h)
    for h in range(2):
        blend_mano(h)
    for i in range(6, NCH):
        skin_chunk(i)

    big_cm.__exit__(None, None, None)
    acc_cm.__exit__(None, None, None)
    es.close()


def _rodrigues(nc, aa, rot, ptile):
    V, S = nc.vector, nc.scalar
    J = NROT
    aa3 = aa[:].rearrange("p (j k) -> p j k", k=3)
    sq = ptile((B, J), "rg_sq")
    tmp = ptile((B, J), "rg_tmp")
    V.tensor_mul(sq[:], aa3[:, :, 0], aa3[:, :, 0])
    V.tensor_mul(tmp[:], aa3[:, :, 1], aa3[:, :, 1])
    V.tensor_add(sq[:], sq[:], tmp[:])
    V.tensor_mul(tmp[:], aa3[:, :, 2], aa3[:, :, 2])
    V.tensor_add(sq[:], sq[:], tmp[:])
    eps_t = ptile((B, 1), "rg_eps")
    nc.gpsimd.memset(eps_t[:], 1e-8)
    hpi_t = ptile((B, 1), "rg_hpi")
    nc.gpsimd.memset(hpi_t[:], float(np.pi / 2))
    zero_t = ptile((B, 1), "rg_zero")
    nc.gpsimd.memset(zero_t[:], 0.0)
    ang = ptile((B, J), "rg_ang")
    S.activation(ang[:], sq[:], AF.Sqrt, bias=eps_t[:])
    inv = ptile((B, J), "rg_inv")
    V.reciprocal(inv[:], ang[:])
    sn = ptile((B, J), "rg_sin")
    co = ptile((B, J), "rg_cos")
    S.activation(sn[:], ang[:], AF.Sin, bias=zero_t[:])
    S.activation(co[:], ang[:], AF.Sin, bias=hpi_t[:])
    nv = ptile((B, 3 * J), "rg_n")
    n3 = nv[:].rearrange("p (j k) -> p j k", k=3)
    V.tensor_mul(n3, aa3, inv[:].unsqueeze(2).broadcast_to([B, J, 3]))
    u = ptile((B, J), "rg_u")
    V.tensor_scalar(u[:], co[:], -1.0, 1.0, ALU.mult, ALU.add)
    un = ptile((B, 3 * J), "rg_un")
    un3 = un[:].rearrange("p (j k) -> p j k", k=3)
    V.tensor_mul(un3, n3, u[:].unsqueeze(2).broadcast_to([B, J, 3]))
    q = ptile((B, 3 * J), "rg_q")
    q3 = q[:].rearrange("p (j k) -> p j k", k=3)
    V.tensor_mul(q3, un3, n3)
    d = ptile((B, J), "rg_d")
    V.tensor_add(d[:], q3[:, :, 0], q3[:, :, 1])
    V.tensor_add(d[:], d[:], q3[:, :, 2])
    dd = ptile((B, J), "rg_dd")
    V.tensor_scalar(dd[:], d[:], -1.0, 1.0, ALU.mult, ALU.add)
    snv = ptile((B, 3 * J), "rg_snv")
    s3 = snv[:].rearrange("p (j k) -> p j k", k=3)
    V.tensor_mul(s3, n3, sn[:].unsqueeze(2).broadcast_to([B, J, 3]))
    r4 = rot[:].rearrange("p (j m n) -> p j m n", m=3, n=3)
    for m in range(3):
        V.tensor_add(r4[:, :, m, m], q3[:, :, m], dd[:])
    p = ptile((B, J), "rg_p")
    V.tensor_mul(p[:], un3[:, :, 0], n3[:, :, 1])
    V.tensor_sub(r4[:, :, 0, 1], p[:], s3[:, :, 2])
    V.tensor_add(r4[:, :, 1, 0], p[:], s3[:, :, 2])
    V.tensor_mul(p[:], un3[:, :, 0], n3[:, :, 2])
    V.tensor_add(r4[:, :, 0, 2], p[:], s3[:, :, 1])
    V.tensor_sub(r4[:, :, 2, 0], p[:], s3[:, :, 1])
    V.tensor_mul(p[:], un3[:, :, 1], n3[:, :, 2])
    V.tensor_sub(r4[:, :, 1, 2], p[:], s3[:, :, 0])
    V.tensor_add(r4[:, :, 2, 1], p[:], s3[:, :, 0])


# ================================================================ entry

_CACHED = {}
DEBUG = False


def _get_nc():
    if "nc" not in _CACHED:
        _CACHED["nc"] = _build_nc()
    return _CACHED["nc"]


PROFILE = False


def kernel(**inputs):
    in_maps, vid_all = _host_prep(inputs)
    nc = _get_nc()
    res = run_bass_kernel_spmd(nc, in_maps, core_ids=list(range(NCORES)),
                               trace=PROFILE)
    _CACHED["last_res"] = res
    out = np.zeros((B, VS, 3), np.float32)
    for c in range(NCORES):
        o = np.asarray(res.results[c]["out"]).reshape(ROWS, 3, B)
        vok = vid_all[c] >= 0
        out[:, vid_all[c][vok], :] = o[vok].transpose(2, 0, 1)
    return out
